# revision 6
# baseline (speedup 1.0000x reference)
"""Expert-choice MoE routing on 8 Trainium2 NeuronCores (Bass/Tile SPMD).

B=8, S=4096, H=2048, E=64, k=640. 8-way token-sharded SPMD; the host
supplies per-core transposed activations xT [H, T_shard] so the logits
matmuls stream xT chunks directly (no on-device X transposes). The
AllToAll probability exchange runs in quarters overlapped under the
phase-1 DMA stream; exact per-expert threshold via branchless bisection.
"""

from contextlib import ExitStack

import concourse.mybir as mybir
from concourse.masks import make_identity
from concourse.tile import TileContext

F32 = mybir.dt.float32
F16 = mybir.dt.float16
I32 = mybir.dt.int32
AX = mybir.AxisListType
OP = mybir.AluOpType
AF = mybir.ActivationFunctionType


def build_kernel(nc, T_shard, H, E, n_cores, k, n_iter):
    assert E == 64 and n_cores == 8
    EPC = E // n_cores          # experts per core = 8
    PPE = 128 // EPC            # count-layout partitions per expert = 16
    T_total = T_shard * n_cores
    TF = T_total // PPE         # tokens per count-layout partition = 2048
    TFH = TF // 2               # DVE half / ACT half of the count pass
    NG = T_shard // 512         # 512-token groups = 8
    NH = H // 128               # contraction chunks = 16
    NT = T_shard // 128         # token tiles = 32
    TQ = T_shard // 4           # exchange quarter = 1024 tokens
    assert T_shard % 2048 == 0 and H % 128 == 0 and TF * PPE == T_total
    # ACT half contributes (s_act + TFH)/2 per partition; over PPE partitions
    # the constant offset is PPE*TFH/2. count >= k <=> est >= k - PPE*TFH/2 - .5
    CMP_GE = float(k) - (PPE * TFH) / 2.0 - 0.5

    xt = nc.dram_tensor("xt", [H, T_shard], F32, kind="ExternalInput")
    wt = nc.dram_tensor("wt", [H, E], F32, kind="ExternalInput")
    probs_o = nc.dram_tensor("probs", [T_shard, E], F32, kind="ExternalOutput")
    disp_o = nc.dram_tensor("disp", [T_shard, E], F32, kind="ExternalOutput")
    comb_o = nc.dram_tensor("comb", [T_shard, E], F32, kind="ExternalOutput")

    with TileContext(nc) as tc, ExitStack() as ctx:
        consts = ctx.enter_context(tc.tile_pool(name="consts", bufs=1))
        persist = ctx.enter_context(tc.tile_pool(name="persist", bufs=1))
        dram = ctx.enter_context(tc.tile_pool(name="dram", bufs=1, space="DRAM"))

        ident = consts.tile([128, 128], F32)
        make_identity(nc, ident[:])

        # ---- data-independent phase-2 constants (built early) ------------
        # expert id of count-layout partition p is (p>>3)&7
        iota_p = consts.tile([128, 1], I32)
        nc.gpsimd.iota(iota_p[:], [[1, 1]], base=0, channel_multiplier=1)
        el_p = consts.tile([128, 1], I32)
        nc.vector.tensor_scalar(el_p[:], iota_p[:], 3, None,
                                op0=OP.arith_shift_right)
        nc.vector.tensor_scalar(el_p[:], el_p[:], EPC - 1, None,
                                op0=OP.bitwise_and)
        iota_f = consts.tile([128, 128], I32)
        nc.gpsimd.iota(iota_f[:], [[1, 128]], base=0, channel_multiplier=0)
        el_f = consts.tile([128, 128], I32)
        nc.vector.tensor_scalar(el_f[:], iota_f[:], 3, None,
                                op0=OP.arith_shift_right)
        nc.vector.tensor_scalar(el_f[:], el_f[:], EPC - 1, None,
                                op0=OP.bitwise_and)
        # expmask[p, p'] = 1.0 if expert(p) == expert(p') (symmetric); fp16
        # halves LDWEIGHTS+MATMUL cost. counts <= 2048 are fp16-exact.
        expmask_f = consts.tile([128, 128], F32)
        nc.vector.tensor_tensor(expmask_f[:], el_p[:].to_broadcast([128, 128]),
                                el_f[:], OP.is_equal)
        expmask = consts.tile([128, 128], F16)
        nc.vector.tensor_copy(expmask[:], expmask_f[:])
        expmask_h = consts.tile([128, 128], F16)
        nc.vector.tensor_scalar_mul(expmask_h[:], expmask_f[:], 0.5)
        # sel8[j, p] = (expert(p) == j) to broadcast [EPC,*] rows to [128,*]
        sel8 = consts.tile([EPC, 128], F32)
        iota_jj = consts.tile([EPC, 1], I32)
        nc.gpsimd.iota(iota_jj[:], [[1, 1]], base=0, channel_multiplier=1)
        el_f8 = consts.tile([EPC, 128], I32)
        nc.gpsimd.iota(el_f8[:], [[1, 128]], base=0, channel_multiplier=0)
        nc.vector.tensor_scalar(el_f8[:], el_f8[:], 3, None,
                                op0=OP.arith_shift_right)
        nc.vector.tensor_scalar(el_f8[:], el_f8[:], EPC - 1, None,
                                op0=OP.bitwise_and)
        nc.vector.tensor_tensor(sel8[:], el_f8[:],
                                iota_jj[:].to_broadcast([EPC, 128]),
                                OP.is_equal)
        signbit = consts.tile([128, 1], I32)
        nc.gpsimd.memset(signbit[:], -2147483648)
        ones1 = consts.tile([1, 128], F32)
        nc.gpsimd.memset(ones1[:], 1.0)

        # router weights, transposed on host: wt [H, E] -> [128, NH, E]
        wt_sb = consts.tile([128, NH, E], F32)
        nc.sync.dma_start(wt_sb[:], wt[:].rearrange("(c p) e -> p c e", p=128))

        # persistent phase-1 results
        probs_sb = persist.tile([128, NT, E], F32)
        probsT_sb = persist.tile([E, T_shard], F32)

        # exchange buffers (token quarters); q3 carries 2 extra columns with
        # this rank's per-expert (max, -min)
        QW = [TQ, TQ, TQ, TQ + 2]
        a2a_in = [dram.tile([E, QW[q]], F32, name=f"a2a_in{q}")
                  for q in range(4)]
        a2a_out = [dram.tile([E, QW[q]], F32, name=f"a2a_out{q}")
                   for q in range(4)]

        p2 = ctx.enter_context(tc.tile_pool(name="p2_sb", bufs=1))
        P_sb = p2.tile([128, TF], F32)
        acc_max = p2.tile([E, 1], F32)
        acc_min = p2.tile([E, 1], F32)

        def exchange_quarter(q):
            nc.sync.dma_start(a2a_in[q][:, 0:TQ],
                              probsT_sb[:, q * TQ:(q + 1) * TQ])
            if q == 3:
                mnmx = p2.tile([E, 2], F32)
                nc.vector.tensor_copy(mnmx[:, 0:1], acc_max[:])
                nc.vector.tensor_scalar_mul(mnmx[:, 1:2], acc_min[:], -1.0)
                nc.sync.dma_start(a2a_in[3][:, TQ:], mnmx[:])
            nc.gpsimd.collective_compute(
                "AllToAll", OP.bypass,
                replica_groups=[list(range(n_cores))],
                ins=[a2a_in[q][:]], outs=[a2a_out[q][:]])
            # count layout: partition p = h*64 + el*8 + r holds tokens
            # [r*T_shard + h*(T_shard//2), +T_shard//2) of expert el, where
            # quarter q covers h = q//2, column half q%2
            h, cq = q // 2, q % 2
            nc.sync.dma_start(
                P_sb[h * 64:(h + 1) * 64, cq * TQ:(cq + 1) * TQ],
                a2a_out[q][:, 0:TQ].rearrange("(r el) t -> el r t", el=EPC))

        # ---- Phase 1 ------------------------------------------------------
        with (
            tc.tile_pool(name="p1_x", bufs=3) as xpool,
            tc.tile_pool(name="p1_sb", bufs=3) as sbpool,
            tc.tile_pool(name="p1_ps_lg", bufs=2, space="PSUM") as ps_lg_pool,
            tc.tile_pool(name="p1_ps_t", bufs=2, space="PSUM") as ps_t_pool,
        ):
            for g in range(NG):
                xb = xpool.tile([128, NH, 512], F32, tag="xb")
                nc.sync.dma_start(
                    xb[:],
                    xt[:, g * 512:(g + 1) * 512].rearrange(
                        "(c p) t -> p c t", p=128))
                ps_lg2 = ps_lg_pool.tile([128, 512], F32, tag="lg")
                for c in range(NH):
                    half = c % 2
                    nc.tensor.matmul(ps_lg2[half * E:(half + 1) * E, :],
                                     wt_sb[:, c, :], xb[:, c, :],
                                     start=(c < 2), stop=(c >= NH - 2),
                                     tile_position=(0, half * E))
                lsumB = sbpool.tile([E, 512], F32, tag="lsumB")
                nc.scalar.copy(lsumB[:], ps_lg2[E:2 * E, :])
                lsum = sbpool.tile([E, 512], F32, tag="lsum")
                nc.vector.tensor_tensor(lsum[:], ps_lg2[0:E, :], lsumB[:],
                                        OP.add)
                exp_sb = sbpool.tile([E, 512], F32, tag="exp")
                nc.scalar.activation(exp_sb[:], lsum[:], AF.Exp)
                ps_eT = ps_t_pool.tile([128, 4, E], F32, tag="t")
                for s in range(4):
                    nc.tensor.transpose(ps_eT[:, s, :],
                                        exp_sb[:, s * 128:(s + 1) * 128],
                                        ident[0:E, 0:E])
                sums = sbpool.tile([128, 4], F32, tag="sums")
                nc.vector.tensor_reduce(sums[:], ps_eT[:], AX.X, OP.add)
                rec = sbpool.tile([128, 4], F32, tag="rec")
                nc.vector.reciprocal(rec[:], sums[:])
                pslice = probs_sb[:, g * 4:(g + 1) * 4, :]
                nc.vector.tensor_tensor(
                    pslice, ps_eT[:],
                    rec[:].rearrange("p (f a) -> p f a", a=1).to_broadcast(
                        [128, 4, E]),
                    OP.mult)
                nc.sync.dma_start(
                    probs_o[g * 512:(g + 1) * 512, :].rearrange(
                        "(s p) e -> p s e", p=128), pslice)
                ps_pT = ps_t_pool.tile([E, 512], F32, tag="t", name="ps_pT")
                for s in range(4):
                    nc.tensor.transpose(ps_pT[:, s * 128:(s + 1) * 128],
                                        probs_sb[:, g * 4 + s, :], ident[:])
                if g % 2 == 0:
                    nc.scalar.copy(probsT_sb[:, g * 512:(g + 1) * 512],
                                   ps_pT[:])
                else:
                    nc.vector.tensor_copy(probsT_sb[:, g * 512:(g + 1) * 512],
                                          ps_pT[:])
                gmax = sbpool.tile([E, 1], F32, tag="gmax")
                nc.vector.tensor_reduce(gmax[:],
                                        probsT_sb[:, g * 512:(g + 1) * 512],
                                        AX.X, OP.max)
                gmin = sbpool.tile([E, 1], F32, tag="gmin")
                nc.vector.tensor_reduce(gmin[:],
                                        probsT_sb[:, g * 512:(g + 1) * 512],
                                        AX.X, OP.min)
                if g == 0:
                    nc.vector.tensor_copy(acc_max[:], gmax[:])
                    nc.vector.tensor_copy(acc_min[:], gmin[:])
                else:
                    nc.vector.tensor_tensor(acc_max[:], acc_max[:], gmax[:],
                                            OP.max)
                    nc.vector.tensor_tensor(acc_min[:], acc_min[:], gmin[:],
                                            OP.min)
                if g % 2 == 1:
                    exchange_quarter(g // 2)

        # ---- Phase 2: branchless threshold bisection ----------------------
        with tc.tile_pool(name="p2_ps", bufs=1, space="PSUM") as p2ps:
            # a2a_out[3] row r*EPC+el, cols [TQ, +2) = rank r's (max, -min)
            mm8 = p2.tile([EPC, n_cores, 2], F32)
            nc.sync.dma_start(
                mm8[:],
                a2a_out[3][:, TQ:].rearrange("(r el) s -> el r s", el=EPC))
            redT_sb = p2.tile([EPC, 2], F32)
            nc.vector.tensor_reduce(redT_sb[:],
                                    mm8[:].rearrange("el r s -> el s r"),
                                    AX.X, OP.max)
            ps_hl = p2ps.tile([128, 2], F32, tag="hl")
            nc.tensor.matmul(ps_hl[:], sel8[:], redT_sb[:], start=True,
                             stop=True)
            lo_f = p2.tile([128, 1], F32)
            hi_f = p2.tile([128, 1], F32)
            nc.vector.tensor_scalar_mul(lo_f[:], ps_hl[:, 1:2], -1.0)
            nc.vector.tensor_scalar_add(hi_f[:], ps_hl[:, 0:1], 0.0)
            lo_i = p2.tile([128, 1], I32)
            hw_i = p2.tile([128, 1], I32)
            step = p2.tile([128, 1], I32)
            nc.vector.tensor_copy(lo_i[:], lo_f[:].bitcast(I32))
            nc.vector.tensor_scalar_add(hw_i[:], hi_f[:].bitcast(I32), 1)
            # step = (hi - lo) >> 1
            nc.vector.tensor_tensor(step[:], hw_i[:], lo_i[:], OP.subtract)
            nc.vector.tensor_scalar(step[:], step[:], 1, None,
                                    op0=OP.arith_shift_right)

            mid_i = p2.tile([128, 1], I32)
            neg_mid = p2.tile([128, 1], I32)
            junk_d = p2.tile([128, TFH], F32)
            junk_a = p2.tile([128, TFH], F32)
            cnt_d = p2.tile([128, 1], F16)
            s_act = p2.tile([128, 1], F16)
            ge = p2.tile([128, 1], I32)
            for it in range(n_iter):
                # mid = lo + step; neg_mid = -bitcast_f32(mid) via sign-bit xor
                nc.vector.tensor_tensor(mid_i[:], lo_i[:], step[:], OP.add)
                nc.vector.tensor_tensor(neg_mid[:], mid_i[:], signbit[:],
                                        OP.bitwise_xor)
                # count(prob >= mid): DVE on first half, ACT sign on second
                with nc.allow_low_precision(
                        reason="counts <= 2048 in integer/half steps are "
                               "fp16-exact"):
                    nc.vector.tensor_scalar(junk_d[:], P_sb[:, 0:TFH],
                                            mid_i[:].bitcast(F32), None,
                                            op0=OP.is_ge, op1=OP.add,
                                            accum_out=cnt_d[:])
                    nc.scalar.activation(junk_a[:], P_sb[:, TFH:TF], AF.Sign,
                                         bias=neg_mid[:].bitcast(F32),
                                         scale=1.0, accum_out=s_act[:])
                ps_cb = p2ps.tile([128, 1], F32, tag="cb")
                nc.tensor.matmul(ps_cb[:], expmask[:], cnt_d[:],
                                 start=True, stop=False)
                nc.tensor.matmul(ps_cb[:], expmask_h[:], s_act[:],
                                 start=False, stop=True)
                nc.vector.tensor_scalar(ge[:], ps_cb[:], CMP_GE, None,
                                        op0=OP.is_ge)
                # lo += ge * step (one fused op); step = (step + 1) >> 1
                nc.vector.scalar_tensor_tensor(lo_i[:], ge[:], step[:],
                                               lo_i[:], op0=OP.mult,
                                               op1=OP.add)
                nc.vector.tensor_scalar_add(step[:], step[:], 1)
                nc.vector.tensor_scalar(step[:], step[:], 1, None,
                                        op0=OP.arith_shift_right)
            # lo lies in (x_{k+1}, x_k]: a valid threshold with count == k
            th_in = dram.tile([128], F32)
            nc.sync.dma_start(th_in[:], lo_i[:].bitcast(F32))
            th_out = dram.tile([128 * n_cores], F32, addr_space="Shared")
            nc.gpsimd.collective_compute(
                "AllGather", OP.bypass,
                replica_groups=[list(range(n_cores))],
                ins=[th_in[:]], outs=[th_out[:]])

        # ---- Phase 3: select, normalize, write out ------------------------
        with (
            tc.tile_pool(name="p3_sb", bufs=1) as p3,
            tc.tile_pool(name="p3_ps", bufs=1, space="PSUM") as p3ps,
        ):
            th_row = consts.tile([1, E], F32)
            # global expert e = r*EPC + el at gathered index r*128 + el*8
            nc.sync.dma_start(
                th_row[:],
                th_out[:].rearrange("(r el s) -> r el s", el=16, s=8)[
                    :, 0:EPC, 0])
            ps_thb = p3ps.tile([128, E], F32)
            nc.tensor.matmul(ps_thb[:], ones1[:], th_row[:], start=True,
                             stop=True)
            th_b = consts.tile([128, E], F32)
            nc.scalar.copy(th_b[:], ps_thb[:])
            NTH = NT // 2
            th_bb = th_b[:].rearrange("p (f e) -> p f e", f=1).to_broadcast(
                [128, NTH, E])
            for hh in range(2):
                fsl = slice(hh * NTH, (hh + 1) * NTH)
                ge_h = p3.tile([128, NTH, E], F32, tag="ge")
                nc.vector.tensor_tensor(ge_h[:], probs_sb[:, fsl, :], th_bb,
                                        OP.is_ge)
                disp_h = p3.tile([128, NTH, E], F32, tag="disp")
                nc.vector.tensor_tensor(disp_h[:], ge_h[:],
                                        probs_sb[:, fsl, :], OP.mult)
                sums_h = p3.tile([128, NTH], F32, tag="sums")
                nc.vector.tensor_reduce(sums_h[:], disp_h[:], AX.X, OP.add)
                nc.vector.tensor_scalar_max(sums_h[:], sums_h[:], 1e-30)
                rec_h = p3.tile([128, NTH], F32, tag="rec")
                nc.vector.reciprocal(rec_h[:], sums_h[:])
                comb_h = p3.tile([128, NTH, E], F32, tag="comb")
                nc.vector.tensor_tensor(
                    comb_h[:], disp_h[:],
                    rec_h[:].rearrange("p (f a) -> p f a", a=1).to_broadcast(
                        [128, NTH, E]),
                    OP.mult)
                # token = f*128 + p in the probs_sb layout
                nc.sync.dma_start(
                    disp_o[hh * NTH * 128:(hh + 1) * NTH * 128, :].rearrange(
                        "(f p) e -> p f e", p=128), disp_h[:])
                nc.sync.dma_start(
                    comb_o[hh * NTH * 128:(hh + 1) * NTH * 128, :].rearrange(
                        "(f p) e -> p f e", p=128), comb_h[:])
    return nc


import numpy as np
import concourse.bacc as bacc
from concourse.bass_utils import run_bass_kernel_spmd

B, S, HH, EE = 8, 4096, 2048, 64
N_CORES = 8
T_TOTAL = B * S
T_SHARD = T_TOTAL // N_CORES
K_CAP = int(1.25 * T_TOTAL / EE)
N_ITER = 19

_NC_CACHE = None


def _get_nc():
    global _NC_CACHE
    if _NC_CACHE is None:
        nc = bacc.Bacc("TRN2", target_bir_lowering=False, debug=False,
                       num_devices=N_CORES)
        build_kernel(nc, T_SHARD, HH, EE, N_CORES, K_CAP, N_ITER)
        nc.compile()
        _NC_CACHE = nc
    return _NC_CACHE


def kernel(hidden_states, router_weight, _trace=False, _trace_cores=None):
    hs = np.ascontiguousarray(np.asarray(hidden_states, dtype=np.float32))
    rw = np.ascontiguousarray(np.asarray(router_weight, dtype=np.float32))
    assert hs.shape == (B, S, HH) and rw.shape == (EE, HH)
    wT = np.ascontiguousarray(rw.T)

    nc = _get_nc()
    in_maps = [
        {"xt": np.ascontiguousarray(hs[c].T), "wt": wT}
        for c in range(N_CORES)
    ]
    res = run_bass_kernel_spmd(
        nc, in_maps, core_ids=list(range(N_CORES)),
        trace=_trace, trace_cores=_trace_cores,
        stitch_traces=bool(_trace_cores and len(_trace_cores) > 1))
    r = res.results

    def gather(name):
        return np.concatenate([r[c][name] for c in range(N_CORES)]).reshape(
            B, S, EE)

    dispatch_mask = gather("disp")
    combine_weights = gather("comb")
    router_probs = gather("probs")
    if _trace:
        kernel.last_exec_time_ns = res.exec_time_ns
        kernel.last_results = res
    return dispatch_mask, combine_weights, router_probs


# revision 16
# speedup vs baseline: 1.0047x; 1.0047x over previous
"""Expert-choice MoE routing on 8 Trainium2 NeuronCores (Bass/Tile SPMD).

B=8, S=4096, H=2048, E=64, k=640. 8-way token-sharded SPMD; the host
supplies per-core transposed activations xT [H, T_shard] so the logits
matmuls stream xT chunks directly (no on-device X transposes). The
AllToAll probability exchange runs in quarters overlapped under the
phase-1 DMA stream; exact per-expert threshold via branchless bisection.
"""

from contextlib import ExitStack

import concourse.mybir as mybir
from concourse.masks import make_identity
from concourse.tile import TileContext

F32 = mybir.dt.float32
F16 = mybir.dt.float16
I32 = mybir.dt.int32
AX = mybir.AxisListType
OP = mybir.AluOpType
AF = mybir.ActivationFunctionType


def build_kernel(nc, T_shard, H, E, n_cores, k, n_iter):
    assert E == 64 and n_cores == 8
    EPC = E // n_cores          # experts per core = 8
    PPE = 128 // EPC            # count-layout partitions per expert = 16
    T_total = T_shard * n_cores
    TF = T_total // PPE         # tokens per count-layout partition = 2048
    TFH = TF // 2               # DVE half / ACT half of the count pass
    NG = T_shard // 512         # 512-token groups = 8
    NH = H // 128               # contraction chunks = 16
    NT = T_shard // 128         # token tiles = 32
    TQ = T_shard // 4           # exchange quarter = 1024 tokens
    assert T_shard % 2048 == 0 and H % 128 == 0 and TF * PPE == T_total
    # ACT half contributes (s_act + TFH)/2 per partition; over PPE partitions
    # the constant offset is PPE*TFH/2. count >= k <=> est >= k - PPE*TFH/2 - .5
    CMP_GE = float(k) - (PPE * TFH) / 2.0 - 0.5

    xt = nc.dram_tensor("xt", [H, T_shard], F32, kind="ExternalInput")
    wt = nc.dram_tensor("wt", [H, E], F32, kind="ExternalInput")
    probs_o = nc.dram_tensor("probs", [T_shard, E], F32, kind="ExternalOutput")
    disp_o = nc.dram_tensor("disp", [T_shard, E], F32, kind="ExternalOutput")
    comb_o = nc.dram_tensor("comb", [T_shard, E], F32, kind="ExternalOutput")

    with TileContext(nc) as tc, ExitStack() as ctx:
        consts = ctx.enter_context(tc.tile_pool(name="consts", bufs=1))
        persist = ctx.enter_context(tc.tile_pool(name="persist", bufs=1))
        dram = ctx.enter_context(tc.tile_pool(name="dram", bufs=1, space="DRAM"))

        ident = consts.tile([128, 128], F32)
        make_identity(nc, ident[:])

        # ---- data-independent phase-2 constants (built early) ------------
        # expert id of count-layout partition p is (p>>3)&7
        iota_p = consts.tile([128, 1], I32)
        nc.gpsimd.iota(iota_p[:], [[1, 1]], base=0, channel_multiplier=1)
        el_p = consts.tile([128, 1], I32)
        nc.vector.tensor_scalar(el_p[:], iota_p[:], 3, None,
                                op0=OP.arith_shift_right)
        nc.vector.tensor_scalar(el_p[:], el_p[:], EPC - 1, None,
                                op0=OP.bitwise_and)
        iota_f = consts.tile([128, 128], I32)
        nc.gpsimd.iota(iota_f[:], [[1, 128]], base=0, channel_multiplier=0)
        el_f = consts.tile([128, 128], I32)
        nc.vector.tensor_scalar(el_f[:], iota_f[:], 3, None,
                                op0=OP.arith_shift_right)
        nc.vector.tensor_scalar(el_f[:], el_f[:], EPC - 1, None,
                                op0=OP.bitwise_and)
        # expmask[p, p'] = 1.0 if expert(p) == expert(p') (symmetric); fp16
        # halves LDWEIGHTS+MATMUL cost. counts <= 2048 are fp16-exact.
        expmask_f = consts.tile([128, 128], F32)
        nc.vector.tensor_tensor(expmask_f[:], el_p[:].to_broadcast([128, 128]),
                                el_f[:], OP.is_equal)
        expmask = consts.tile([128, 128], F16)
        nc.vector.tensor_copy(expmask[:], expmask_f[:])
        expmask_h = consts.tile([128, 128], F16)
        nc.vector.tensor_scalar_mul(expmask_h[:], expmask_f[:], 0.5)
        # sel8[j, p] = (expert(p) == j) to broadcast [EPC,*] rows to [128,*]
        sel8 = consts.tile([EPC, 128], F32)
        iota_jj = consts.tile([EPC, 1], I32)
        nc.gpsimd.iota(iota_jj[:], [[1, 1]], base=0, channel_multiplier=1)
        el_f8 = consts.tile([EPC, 128], I32)
        nc.gpsimd.iota(el_f8[:], [[1, 128]], base=0, channel_multiplier=0)
        nc.vector.tensor_scalar(el_f8[:], el_f8[:], 3, None,
                                op0=OP.arith_shift_right)
        nc.vector.tensor_scalar(el_f8[:], el_f8[:], EPC - 1, None,
                                op0=OP.bitwise_and)
        nc.vector.tensor_tensor(sel8[:], el_f8[:],
                                iota_jj[:].to_broadcast([EPC, 128]),
                                OP.is_equal)
        signbit = consts.tile([128, 1], I32)
        nc.gpsimd.memset(signbit[:], -2147483648)
        ones1 = consts.tile([1, 128], F32)
        nc.gpsimd.memset(ones1[:], 1.0)

        # router weights, transposed on host: wt [H, E] -> [128, NH, E]
        wt_sb = consts.tile([128, NH, E], F32)
        nc.sync.dma_start(wt_sb[:], wt[:].rearrange("(c p) e -> p c e", p=128))

        # persistent phase-1 results
        probs_sb = persist.tile([128, NT, E], F32)
        probsT_sb = persist.tile([E, T_shard], F32)

        # exchange buffers (token quarters); q3 carries 2 extra columns with
        # this rank's per-expert (max, -min)
        HW_ = [T_shard // 2, T_shard // 2 + 2]
        a2a_in = [dram.tile([E, HW_[h]], F32, name=f"a2a_in{h}")
                  for h in range(2)]
        a2a_out = [dram.tile([E, HW_[h]], F32, name=f"a2a_out{h}")
                   for h in range(2)]

        p2 = ctx.enter_context(tc.tile_pool(name="p2_sb", bufs=1))
        P_sb = p2.tile([128, TF], F32)
        acc_max = p2.tile([E, 1], F32)
        acc_min = p2.tile([E, 1], F32)

        def exchange_half(h):
            nc.sync.dma_start(
                a2a_in[h][:, 0:T_shard // 2],
                probsT_sb[:, h * (T_shard // 2):(h + 1) * (T_shard // 2)])
            if h == 1:
                mnmx = p2.tile([E, 2], F32)
                nc.vector.tensor_copy(mnmx[:, 0:1], acc_max[:])
                nc.vector.tensor_scalar_mul(mnmx[:, 1:2], acc_min[:], -1.0)
                nc.sync.dma_start(a2a_in[1][:, T_shard // 2:], mnmx[:])
            nc.gpsimd.collective_compute(
                "AllToAll", OP.bypass,
                replica_groups=[list(range(n_cores))],
                ins=[a2a_in[h][:]], outs=[a2a_out[h][:]])
            # count layout: partition p = h*64 + el*8 + r holds tokens
            # [r*T_shard + h*(T_shard//2), +T_shard//2) of this core's expert el
            nc.sync.dma_start(
                P_sb[h * 64:(h + 1) * 64, :],
                a2a_out[h][:, 0:T_shard // 2].rearrange("(r el) t -> el r t",
                                                        el=EPC))

        # ---- Phase 1 ------------------------------------------------------
        with (
            tc.tile_pool(name="p1_x", bufs=3) as xpool,
            tc.tile_pool(name="p1_sb", bufs=3) as sbpool,
            tc.tile_pool(name="p1_ps_lg", bufs=2, space="PSUM") as ps_lg_pool,
            tc.tile_pool(name="p1_ps_t", bufs=2, space="PSUM") as ps_t_pool,
        ):
            for g in range(NG):
                xb = xpool.tile([128, NH, 512], F32, tag="xb")
                if g == 0:
                    # split the first load so matmuls start sooner
                    for ch in range(2):
                        nc.sync.dma_start(
                            xb[:, ch * 8:(ch + 1) * 8, :],
                            xt[ch * 1024:(ch + 1) * 1024,
                               0:512].rearrange("(c p) t -> p c t", p=128))
                else:
                    nc.sync.dma_start(
                        xb[:],
                        xt[:, g * 512:(g + 1) * 512].rearrange(
                            "(c p) t -> p c t", p=128))
                ps_lg2 = ps_lg_pool.tile([128, 512], F32, tag="lg")
                for c in range(NH):
                    half = c % 2
                    nc.tensor.matmul(ps_lg2[half * E:(half + 1) * E, :],
                                     wt_sb[:, c, :], xb[:, c, :],
                                     start=(c < 2), stop=(c >= NH - 2),
                                     tile_position=(0, half * E))
                lsumB = sbpool.tile([E, 512], F32, tag="lsumB")
                nc.scalar.copy(lsumB[:], ps_lg2[E:2 * E, :])
                lsum = sbpool.tile([E, 512], F32, tag="lsum")
                nc.vector.tensor_tensor(lsum[:], ps_lg2[0:E, :], lsumB[:],
                                        OP.add)
                exp_sb = sbpool.tile([E, 512], F32, tag="exp")
                nc.scalar.activation(exp_sb[:], lsum[:], AF.Exp)
                ps_eT = ps_t_pool.tile([128, 4, E], F32, tag="t")
                for s in range(4):
                    nc.tensor.transpose(ps_eT[:, s, :],
                                        exp_sb[:, s * 128:(s + 1) * 128],
                                        ident[0:E, 0:E])
                sums = sbpool.tile([128, 4], F32, tag="sums")
                nc.vector.tensor_reduce(sums[:], ps_eT[:], AX.X, OP.add)
                rec = sbpool.tile([128, 4], F32, tag="rec")
                nc.vector.reciprocal(rec[:], sums[:])
                pslice = probs_sb[:, g * 4:(g + 1) * 4, :]
                nc.vector.tensor_tensor(
                    pslice, ps_eT[:],
                    rec[:].rearrange("p (f a) -> p f a", a=1).to_broadcast(
                        [128, 4, E]),
                    OP.mult)
                ps_pT = ps_t_pool.tile([E, 512], F32, tag="t", name="ps_pT")
                for s in range(4):
                    nc.tensor.transpose(ps_pT[:, s * 128:(s + 1) * 128],
                                        probs_sb[:, g * 4 + s, :], ident[:])
                if g % 2 == 0:
                    nc.scalar.copy(probsT_sb[:, g * 512:(g + 1) * 512],
                                   ps_pT[:])
                else:
                    nc.vector.tensor_copy(probsT_sb[:, g * 512:(g + 1) * 512],
                                          ps_pT[:])
                gmax = sbpool.tile([E, 1], F32, tag="gmax")
                nc.vector.tensor_reduce(gmax[:],
                                        probsT_sb[:, g * 512:(g + 1) * 512],
                                        AX.X, OP.max)
                gmin = sbpool.tile([E, 1], F32, tag="gmin")
                nc.vector.tensor_reduce(gmin[:],
                                        probsT_sb[:, g * 512:(g + 1) * 512],
                                        AX.X, OP.min)
                if g == 0:
                    nc.vector.tensor_copy(acc_max[:], gmax[:])
                    nc.vector.tensor_copy(acc_min[:], gmin[:])
                else:
                    nc.vector.tensor_tensor(acc_max[:], acc_max[:], gmax[:],
                                            OP.max)
                    nc.vector.tensor_tensor(acc_min[:], acc_min[:], gmin[:],
                                            OP.min)
                if g == NG // 2 - 1:
                    exchange_half(0)
            exchange_half(1)
            # probs output rides the DMA-idle window during the bisection
            nc.sync.dma_start(
                probs_o[:].rearrange("(f p) e -> p f e", p=128), probs_sb[:])

        # ---- Phase 2: branchless threshold bisection ----------------------
        with tc.tile_pool(name="p2_ps", bufs=1, space="PSUM") as p2ps:
            # a2a_out[1] row r*EPC+el, cols [T_shard//2, +2) = rank r's
            # (max, -min) for this core's expert el
            mm8 = p2.tile([EPC, n_cores, 2], F32)
            nc.sync.dma_start(
                mm8[:],
                a2a_out[1][:, T_shard // 2:].rearrange(
                    "(r el) s -> el r s", el=EPC))
            redT_sb = p2.tile([EPC, 2], F32)
            nc.vector.tensor_reduce(redT_sb[:],
                                    mm8[:].rearrange("el r s -> el s r"),
                                    AX.X, OP.max)
            ps_hl = p2ps.tile([128, 2], F32, tag="hl")
            nc.tensor.matmul(ps_hl[:], sel8[:], redT_sb[:], start=True,
                             stop=True)
            lo_f = p2.tile([128, 1], F32)
            hi_f = p2.tile([128, 1], F32)
            nc.vector.tensor_scalar_mul(lo_f[:], ps_hl[:, 1:2], -1.0)
            nc.vector.tensor_scalar_add(hi_f[:], ps_hl[:, 0:1], 0.0)
            lo_i = p2.tile([128, 1], I32)
            hw_i = p2.tile([128, 1], I32)
            step = p2.tile([128, 1], I32)
            nc.vector.tensor_copy(lo_i[:], lo_f[:].bitcast(I32))
            nc.vector.tensor_scalar_add(hw_i[:], hi_f[:].bitcast(I32), 1)
            # step = (hi - lo) >> 1
            nc.vector.tensor_tensor(step[:], hw_i[:], lo_i[:], OP.subtract)
            nc.vector.tensor_scalar(step[:], step[:], 1, None,
                                    op0=OP.arith_shift_right)

            mid_i = p2.tile([128, 1], I32)
            neg_mid = p2.tile([128, 1], I32)
            junk_d = p2.tile([128, TFH], F32)
            junk_a = p2.tile([128, TFH], F32)
            cnt_d = p2.tile([128, 1], F16)
            s_act = p2.tile([128, 1], F16)
            ge = p2.tile([128, 1], I32)
            for it in range(n_iter):
                # mid = lo + step; neg_mid = -bitcast_f32(mid) via sign-bit xor
                nc.vector.tensor_tensor(mid_i[:], lo_i[:], step[:], OP.add)
                nc.vector.tensor_tensor(neg_mid[:], mid_i[:], signbit[:],
                                        OP.bitwise_xor)
                # count(prob >= mid): DVE on first half, ACT sign on second
                with nc.allow_low_precision(
                        reason="counts <= 2048 in integer/half steps are "
                               "fp16-exact"):
                    nc.vector.tensor_scalar(junk_d[:], P_sb[:, 0:TFH],
                                            mid_i[:].bitcast(F32), None,
                                            op0=OP.is_ge, op1=OP.add,
                                            accum_out=cnt_d[:])
                    nc.scalar.activation(junk_a[:], P_sb[:, TFH:TF], AF.Sign,
                                         bias=neg_mid[:].bitcast(F32),
                                         scale=1.0, accum_out=s_act[:])
                ps_cb = p2ps.tile([128, 1], F32, tag="cb")
                nc.tensor.matmul(ps_cb[:], expmask[:], cnt_d[:],
                                 start=True, stop=False)
                nc.tensor.matmul(ps_cb[:], expmask_h[:], s_act[:],
                                 start=False, stop=True)
                nc.vector.tensor_scalar(ge[:], ps_cb[:], CMP_GE, None,
                                        op0=OP.is_ge)
                # lo += ge * step (one fused op); step = (step + 1) >> 1
                nc.vector.scalar_tensor_tensor(lo_i[:], ge[:], step[:],
                                               lo_i[:], op0=OP.mult,
                                               op1=OP.add)
                nc.vector.tensor_scalar_add(step[:], step[:], 1)
                nc.vector.tensor_scalar(step[:], step[:], 1, None,
                                        op0=OP.arith_shift_right)
            # lo lies in (x_{k+1}, x_k]: a valid threshold with count == k
            th_in = dram.tile([128], F32)
            nc.gpsimd.dma_start(th_in[:], lo_i[:].bitcast(F32))
            th_out = dram.tile([128 * n_cores], F32, addr_space="Shared")
            nc.gpsimd.collective_compute(
                "AllGather", OP.bypass,
                replica_groups=[list(range(n_cores))],
                ins=[th_in[:]], outs=[th_out[:]])

        # ---- Phase 3: select, normalize, write out ------------------------
        with (
            tc.tile_pool(name="p3_sb", bufs=1) as p3,
            tc.tile_pool(name="p3_ps", bufs=1, space="PSUM") as p3ps,
        ):
            th_row = consts.tile([1, E], F32)
            # global expert e = r*EPC + el at gathered index r*128 + el*8
            nc.sync.dma_start(
                th_row[:],
                th_out[:].rearrange("(r el s) -> r el s", el=16, s=8)[
                    :, 0:EPC, 0])
            ps_thb = p3ps.tile([128, E], F32)
            nc.tensor.matmul(ps_thb[:], ones1[:], th_row[:], start=True,
                             stop=True)
            th_b = consts.tile([128, E], F32)
            nc.scalar.copy(th_b[:], ps_thb[:])
            NTH = NT // 2
            th_bb = th_b[:].rearrange("p (f e) -> p f e", f=1).to_broadcast(
                [128, NTH, E])
            # DVE: compares + reduces + reciprocals; Pool: the mult ops it
            # supports (is_ge is not a Pool ALU op, nor free-axis reduce)
            fsl = [slice(hh * NTH, (hh + 1) * NTH) for hh in range(2)]
            ge_t = [p3.tile([128, NTH, E], F32, tag=f"ge{hh}",
                            name=f"ge{hh}") for hh in range(2)]
            disp_t = [p3.tile([128, NTH, E], F32, tag=f"disp{hh}",
                              name=f"disp{hh}") for hh in range(2)]
            sums_t = [p3.tile([128, NTH], F32, tag=f"sums{hh}",
                              name=f"sums{hh}") for hh in range(2)]
            rec_t = [p3.tile([128, NTH], F32, tag=f"rec{hh}",
                             name=f"rec{hh}") for hh in range(2)]
            comb_t = [p3.tile([128, NTH, E], F32, tag=f"comb{hh}",
                              name=f"comb{hh}") for hh in range(2)]

            def out_dma(dst, hh, src):
                nc.sync.dma_start(
                    dst[hh * NTH * 128:(hh + 1) * NTH * 128, :].rearrange(
                        "(f p) e -> p f e", p=128), src[:])

            rec_b = [rec_t[hh][:].rearrange("p (f a) -> p f a",
                                            a=1).to_broadcast([128, NTH, E])
                     for hh in range(2)]
            nc.vector.tensor_tensor(ge_t[0][:], probs_sb[:, fsl[0], :], th_bb,
                                    OP.is_ge)
            nc.vector.tensor_tensor(disp_t[0][:], ge_t[0][:],
                                    probs_sb[:, fsl[0], :], OP.mult)
            nc.vector.tensor_tensor(ge_t[1][:], probs_sb[:, fsl[1], :], th_bb,
                                    OP.is_ge)
            nc.gpsimd.tensor_tensor(disp_t[1][:], ge_t[1][:],
                                    probs_sb[:, fsl[1], :], OP.mult)
            nc.vector.tensor_reduce(sums_t[0][:], disp_t[0][:], AX.X, OP.add)
            nc.vector.tensor_scalar_max(sums_t[0][:], sums_t[0][:], 1e-30)
            nc.vector.reciprocal(rec_t[0][:], sums_t[0][:])
            nc.gpsimd.tensor_tensor(comb_t[0][:], disp_t[0][:], rec_b[0],
                                    OP.mult)
            out_dma(disp_o, 0, disp_t[0])
            nc.vector.tensor_reduce(sums_t[1][:], disp_t[1][:], AX.X, OP.add)
            nc.vector.tensor_scalar_max(sums_t[1][:], sums_t[1][:], 1e-30)
            nc.vector.reciprocal(rec_t[1][:], sums_t[1][:])
            out_dma(disp_o, 1, disp_t[1])
            nc.vector.tensor_tensor(comb_t[1][:], disp_t[1][:], rec_b[1],
                                    OP.mult)
            out_dma(comb_o, 0, comb_t[0])
            out_dma(comb_o, 1, comb_t[1])
    return nc


import numpy as np
import concourse.bacc as bacc
from concourse.bass_utils import run_bass_kernel_spmd

B, S, HH, EE = 8, 4096, 2048, 64
N_CORES = 8
T_TOTAL = B * S
T_SHARD = T_TOTAL // N_CORES
K_CAP = int(1.25 * T_TOTAL / EE)
N_ITER = 19

_NC_CACHE = None


def _get_nc():
    global _NC_CACHE
    if _NC_CACHE is None:
        nc = bacc.Bacc("TRN2", target_bir_lowering=False, debug=False,
                       num_devices=N_CORES)
        build_kernel(nc, T_SHARD, HH, EE, N_CORES, K_CAP, N_ITER)
        nc.compile()
        _NC_CACHE = nc
    return _NC_CACHE


def kernel(hidden_states, router_weight, _trace=False, _trace_cores=None):
    hs = np.ascontiguousarray(np.asarray(hidden_states, dtype=np.float32))
    rw = np.ascontiguousarray(np.asarray(router_weight, dtype=np.float32))
    assert hs.shape == (B, S, HH) and rw.shape == (EE, HH)
    wT = np.ascontiguousarray(rw.T)

    nc = _get_nc()
    in_maps = [
        {"xt": np.ascontiguousarray(hs[c].T), "wt": wT}
        for c in range(N_CORES)
    ]
    res = run_bass_kernel_spmd(
        nc, in_maps, core_ids=list(range(N_CORES)),
        trace=_trace, trace_cores=_trace_cores,
        stitch_traces=bool(_trace_cores and len(_trace_cores) > 1))
    r = res.results

    def gather(name):
        return np.concatenate([r[c][name] for c in range(N_CORES)]).reshape(
            B, S, EE)

    dispatch_mask = gather("disp")
    combine_weights = gather("comb")
    router_probs = gather("probs")
    if _trace:
        kernel.last_exec_time_ns = res.exec_time_ns
        kernel.last_results = res
    return dispatch_mask, combine_weights, router_probs
